# revision 1
# baseline (speedup 1.0000x reference)
"""Trainium2 Bass kernel for GroupedKAAttention.

Math (per batch row b of B=4096, fp32 reference):
  xg[b,g,:]  = x[b, g*64:(g+1)*64]                      (G=64 groups, D=64)
  h[b,g,:]   = silu(xg[b,g,:] @ W1[g] + b1[g])          (H=512)
  f[b,g,:]   = h[b,g,:] @ W2[g] + b2[g]                 (P=64 patches)
  h2[b,p,:]  = silu(f[b,:,p] @ Wg1 + bg1)               (contract groups)
  o[b,p,:]   = h2[b,p,:] @ Wg2 + bg2                    (E=16 heads)
  attn[b]    = sum_{p,e} o_q * o_k ;  out = softmax(attn over b)

Distribution: data-parallel over batch, 512 rows per core, weights
replicated.  Activations live feature-on-partition / batch-on-free.
Matmuls run in fp32r (~1.6e-4 rel err, full PE speed at N=512; fp32
PSUM accumulate).  The grouped-stage bias is baked in as a 65th
contraction row; SiLU runs on the scalar engine while draining PSUM
(that engine is the critical path: ~67M evals/core).  The (g,p)
transpose between grouped and global stages is a DRAM bounce with
strided DMA access patterns.  The global stage row-tiles patch pairs
(fp32r forbids column tile_position).  Per-core output is 512 attention
logits; softmax over the full 4096 batch is applied on host.
"""

import numpy as np

B = 4096
TOTAL_DIM = 4096
G = 64            # groups
D = 64            # group size
H = 512           # hidden
P = 64            # patches
E = 16            # heads
NCORES = 8
BC = B // NCORES  # 512 batch rows per core
NPAIR = P // 2    # 32 patch pairs (global stage)


def _build_nc():
    from contextlib import ExitStack
    import concourse.bass as bass
    import concourse.tile as tile
    import concourse.mybir as mybir
    from concourse import bacc

    dt = mybir.dt
    fr = dt.float32r
    f32 = dt.float32
    AF = mybir.ActivationFunctionType

    nc = bacc.Bacc(
        "TRN2",
        target_bir_lowering=False,
        debug=False,
        enable_asserts=False,
        num_devices=NCORES,
    )

    ins = {}
    def din(name, shape, dty):
        ins[name] = nc.dram_tensor(name, shape, dty, kind="ExternalInput").ap()
        return ins[name]

    xq = din("xq", [G * (D + 1), BC], fr)      # rows g*65+d (d<64: x^T), row 64: ones
    xk = din("xk", [G * (D + 1), BC], fr)
    w1q = din("w1q", [G * (D + 1), H], fr)     # rows g*65+d: W1[g,d,:], row 64: b1[g]
    w1k = din("w1k", [G * (D + 1), H], fr)
    w2q = din("w2q", [G * 128, 4 * 64], fr)    # group g rows: [r, hc*64+p] = W2[g, hc*128+r, p]
    w2k = din("w2k", [G * 128, 4 * 64], fr)
    wg1 = din("wg1", [128, H], fr)             # Wg1 [64,512] duplicated on both partition halves
    wg2 = din("wg2", [128, 4 * 32], fr)        # [r, hc*32+e] = Wg2[hc*128+r, e] (e<16, else 0)
    b2q = din("b2q", [64, G], f32)             # col g = b2[g]
    b2k = din("b2k", [64, G], f32)
    bg1p = din("bg1p", [128, 4], f32)          # col hc = bg1[hc*128:(hc+1)*128]
    bg2r = din("bg2r", [128, 1], f32)          # 4x [bg2(16); zeros(16)] along partitions
    ones128 = din("ones128", [128, 1], fr)

    out = nc.dram_tensor("out", [1, BC], f32, kind="ExternalOutput").ap()

    with tile.TileContext(nc) as tc:
        with ExitStack() as ctx:
            ep = ctx.enter_context
            px = ep(tc.tile_pool(name="px", bufs=6))          # x tiles [65,BC]
            pw1 = ep(tc.tile_pool(name="pw1", bufs=6))        # W1 tiles [65,H]
            pw2 = ep(tc.tile_pool(name="pw2", bufs=4))        # W2 group tiles [128,256]
            phs = ep(tc.tile_pool(name="phs", bufs=4))        # silu'd h [128,1024]
            pfv = ep(tc.tile_pool(name="pfv", bufs=4))        # f group tiles [64,BC]
            pu = ep(tc.tile_pool(name="pu", bufs=6))          # U tiles [128,BC]
            ph2 = ep(tc.tile_pool(name="ph2", bufs=10))       # silu'd h2 [128,1024]
            pbig = ep(tc.tile_pool(name="pbig", bufs=1))      # qs/ks/prod [128,8*BC]
            pmisc = ep(tc.tile_pool(name="pmisc", bufs=2))
            pconst = ep(tc.tile_pool(name="pconst", bufs=1))
            # PSUM: psh 3 x 2 banks + psv 2 x 1 bank = 8 banks
            psh = ep(tc.tile_pool(name="psh", bufs=3, space="PSUM"))
            psv = ep(tc.tile_pool(name="psv", bufs=2, space="PSUM"))
            pdram = ep(tc.tile_pool(name="pdram", bufs=1, space="DRAM"))

            def const_tile(src_ap, shape, dty, name):
                t = pconst.tile(shape, dty, name=name, tag=name)
                nc.sync.dma_start(t[:, :], src_ap)
                return t

            wg1_s = const_tile(wg1, [128, H], fr, "wg1s")
            wg2_s = const_tile(wg2, [128, 4 * 32], fr, "wg2s")
            b2q_s = const_tile(b2q, [64, G], f32, "b2qs")
            b2k_s = const_tile(b2k, [64, G], f32, "b2ks")
            bg1_s = const_tile(bg1p, [128, 4], f32, "bg1s")
            bg2_s = const_tile(bg2r, [128, 1], f32, "bg2s")
            one_s = const_tile(ones128, [128, 1], fr, "ones")

            f_dram = {
                "q": pdram.tile([G * P, BC], fr, name="fq", tag="fq"),
                "k": pdram.tile([G * P, BC], fr, name="fk", tag="fk"),
            }
            stream_in = {"q": (xq, w1q, w2q, b2q_s), "k": (xk, w1k, w2k, b2k_s)}

            # ================= grouped stage =================
            def grouped(s):
                x_d, w1_d, w2_d, b2_s = stream_in[s]
                fd = f_dram[s]
                for g in range(G):
                    x_t = px.tile([D + 1, BC], fr, tag="x")
                    nc.sync.dma_start(x_t[:, :], x_d[g * 65:(g + 1) * 65, :])
                    w1_t = pw1.tile([D + 1, H], fr, tag="w1")
                    nc.sync.dma_start(w1_t[:, :], w1_d[g * 65:(g + 1) * 65, :])
                    w2_t = pw2.tile([128, 4 * 64], fr, tag="w2")
                    nc.sync.dma_start(w2_t[:, :], w2_d[g * 128:(g + 1) * 128, :])
                    v_ps = psv.tile([64, BC], f32, tag="vps")
                    for t in range(2):       # two [128,1024] PSUM tiles = 4 h-chunks
                        hp = psh.tile([128, 1024], f32, tag="hps")
                        for u in range(2):
                            hc = 2 * t + u
                            nc.tensor.matmul(
                                hp[:, u * 512:(u + 1) * 512],
                                w1_t[:, hc * 128:(hc + 1) * 128],
                                x_t[:, :],
                                start=True, stop=True,
                            )
                        hs_t = phs.tile([128, 1024], fr, tag="hs")
                        nc.scalar.activation(hs_t[:, :], hp[:, :], AF.Silu)
                        for u in range(2):   # GEMM2 accumulation
                            hc = 2 * t + u
                            nc.tensor.matmul(
                                v_ps[:, :],
                                w2_t[:, hc * 64:(hc + 1) * 64],
                                hs_t[:, u * 512:(u + 1) * 512],
                                start=(hc == 0), stop=(hc == 3),
                            )
                    fv = pfv.tile([64, BC], fr, tag="fv")
                    nc.vector.tensor_scalar_add(fv[:, :], v_ps[:, :], b2_s[:, g:g + 1])
                    nc.sync.dma_start(fd[g * 64:(g + 1) * 64, :], fv[:, :])

            # ================= global stage =================
            def global_stream(s, big):
                fd3 = f_dram[s].rearrange("(g p) b -> p g b", p=P)
                for j in range(NPAIR):       # patch pair (2j, 2j+1)
                    u_t = pu.tile([128, BC], fr, tag="u")
                    nc.sync.dma_start(u_t[:, :], fd3[2 * j:2 * j + 2])
                    h2s = []
                    for hc in range(4):
                        h2p = psh.tile([128, 1024], f32, tag="hps")
                        for dp in range(2):
                            nc.tensor.matmul(
                                h2p[:, dp * 512:(dp + 1) * 512],
                                wg1_s[dp * 64:(dp + 1) * 64, hc * 128:(hc + 1) * 128],
                                u_t[dp * 64:(dp + 1) * 64, :],
                                start=True, stop=True,
                                tile_position=(dp * 64, 0),
                            )
                        t = ph2.tile([128, 1024], fr, tag="h2s")
                        nc.scalar.activation(t[:, :], h2p[:, :], AF.Silu,
                                             bias=bg1_s[:, hc:hc + 1])
                        h2s.append(t)
                    for dp in range(2):      # head GEMM per patch (M=32, top 16 real)
                        p_ = 2 * j + dp
                        o_ps = psv.tile([32, BC], f32, tag="vps")
                        for hc in range(4):
                            nc.tensor.matmul(
                                o_ps[:, :],
                                wg2_s[:, hc * 32:(hc + 1) * 32],
                                h2s[hc][:, dp * 512:(dp + 1) * 512],
                                start=(hc == 0), stop=(hc == 3),
                            )
                        # drain into big [128, 16*BC]: partition 32*(p%4), col-block p//4
                        pr, pcb = 32 * (p_ % 4), (p_ // 4) * BC
                        nc.vector.tensor_scalar_add(
                            big[pr:pr + 32, pcb:pcb + BC], o_ps[:, :],
                            bg2_s[pr:pr + 32, 0:1])

            grouped("q")
            grouped("k")

            qs_big = pbig.tile([128, 16 * BC], f32, tag="qsbig")
            ks_big = pbig.tile([128, 16 * BC], f32, tag="ksbig")
            global_stream("q", qs_big)
            global_stream("k", ks_big)

            # ============ dot product + logits ============
            prod = ks_big   # in-place q*k
            nc.vector.tensor_mul(prod[:, :], qs_big[:, :], ks_big[:, :])
            red = pmisc.tile([128, BC], fr, tag="red")
            with nc.allow_low_precision(reason="fp32r reduce of 8 fp32 blocks"):
                nc.vector.tensor_reduce(
                    red[:, :],
                    prod[:, :].rearrange("a (c b) -> a b c", b=BC),
                    axis=mybir.AxisListType.X,
                    op=mybir.AluOpType.add,
                )
            at_ps = psv.tile([1, BC], f32, tag="vps")
            nc.tensor.matmul(at_ps[0:1, :], one_s[:, 0:1], red[:, :],
                             start=True, stop=True)
            at_s = pmisc.tile([1, BC], f32, tag="at")
            nc.vector.tensor_copy(at_s[0:1, :], at_ps[0:1, :])
            nc.sync.dma_start(out[0:1, :], at_s[0:1, :])

    nc.compile()
    return nc


_NC_CACHE = None


def _get_nc():
    global _NC_CACHE
    if _NC_CACHE is None:
        _NC_CACHE = _build_nc()
    return _NC_CACHE


def _prep_inputs(q, k, W1q, b1q, W2q, b2q, W1k, b1k, W2k, b2k, Wg1, bg1, Wg2, bg2):
    f32c = lambda a: np.ascontiguousarray(a, dtype=np.float32)

    def pack_x(x):  # [B, 4096] -> per-core [G*65, BC] with ones row
        shards = []
        for c in range(NCORES):
            xs = x[c * BC:(c + 1) * BC, :]
            xt = np.empty((G, D + 1, BC), dtype=np.float32)
            xt[:, :D, :] = xs.T.reshape(G, D, BC)
            xt[:, D, :] = 1.0
            shards.append(f32c(xt.reshape(G * (D + 1), BC)))
        return shards

    def pack_w1(W1, b1):
        w = np.concatenate([np.asarray(W1, np.float32),
                            np.asarray(b1, np.float32)[:, None, :]], axis=1)
        return f32c(w.reshape(G * (D + 1), H))

    def pack_w2(W2):  # [G, 512, 64] -> [G*128, 4*64]
        w = np.asarray(W2, np.float32).reshape(G, 4, 128, 64)   # [g, hc, r, p]
        w = w.transpose(0, 2, 1, 3)                             # [g, r, hc, p]
        return f32c(w.reshape(G * 128, 4 * 64))

    xq_s = pack_x(q)
    xk_s = pack_x(k)
    w1q_p = pack_w1(W1q, b1q)
    w1k_p = pack_w1(W1k, b1k)
    w2q_p = pack_w2(W2q)
    w2k_p = pack_w2(W2k)
    b2q_p = f32c(np.asarray(b2q, np.float32).T)                 # [64(P), G]
    b2k_p = f32c(np.asarray(b2k, np.float32).T)

    wg1_p = f32c(np.concatenate([Wg1, Wg1], axis=0))            # [128, 512]
    wg2_p = np.zeros((128, 4, 32), dtype=np.float32)
    wg2_p[:, :, :E] = np.asarray(Wg2, np.float32).reshape(4, 128, E).transpose(1, 0, 2)
    wg2_p = f32c(wg2_p.reshape(128, 4 * 32))                    # [r, hc*32+e]
    bg1_p = f32c(np.asarray(bg1, np.float32).reshape(4, 128).T)  # [128, 4]
    bg2_p = np.zeros((4, 32), dtype=np.float32)
    bg2_p[:, :E] = np.asarray(bg2, np.float32)
    bg2_p = f32c(bg2_p.reshape(128, 1))
    ones_p = np.ones((128, 1), dtype=np.float32)

    in_maps = []
    for c in range(NCORES):
        in_maps.append({
            "xq": xq_s[c], "xk": xk_s[c],
            "w1q": w1q_p, "w1k": w1k_p,
            "w2q": w2q_p, "w2k": w2k_p,
            "wg1": wg1_p, "wg2": wg2_p,
            "b2q": b2q_p, "b2k": b2k_p,
            "bg1p": bg1_p, "bg2r": bg2_p, "ones128": ones_p,
        })
    return in_maps


def kernel(q, k, W1q, b1q, W2q, b2q, W1k, b1k, W2k, b2k, Wg1, bg1, Wg2, bg2,
           _trace=False, _tracedir=None):
    from concourse.bass_utils import run_bass_kernel_spmd

    in_maps = _prep_inputs(q, k, W1q, b1q, W2q, b2q, W1k, b1k, W2k, b2k,
                           Wg1, bg1, Wg2, bg2)
    nc = _get_nc()
    kw = {}
    if _trace:
        kw = dict(trace=True, tmpdir=_tracedir)
    res = run_bass_kernel_spmd(nc, in_maps, core_ids=list(range(NCORES)), **kw)
    logits = np.concatenate([res.results[c]["out"].reshape(BC)
                             for c in range(NCORES)]).astype(np.float64)
    m = logits.max()
    e = np.exp(logits - m)
    sm = (e / e.sum()).astype(np.float32)
    if _trace:
        kernel._last_trace = res
    return sm



# revision 2
# speedup vs baseline: 8244.6618x; 8244.6618x over previous
"""Trainium2 Bass kernel for GroupedKAAttention, v2 (fp16 datapath).

Math per batch row b (B=4096 total, 512 per core):
  xg[b,g,:]  = x[b, g*64:(g+1)*64]                      (G=64 groups, D=64)
  h[b,g,:]   = silu(xg[b,g,:] @ W1[g] + b1[g])          (H=512)
  f[b,g,:]   = h[b,g,:] @ W2[g] + b2[g]                 (P=64 patches)
  h2[b,p,:]  = silu(f[b,:,p] @ Wg1 + bg1)               (contract groups)
  o[b,p,:]   = h2[b,p,:] @ Wg2 + bg2                    (E=16 heads)
  attn[b]    = sum_{p,e} o_q * o_k ;  out = softmax(attn over b)

v2 vs v1: the whole DMA path runs fp16 (x, W1, W2, f, Wg1, Wg2,
activations) — PE fp16 matmuls run at the same 1 cycle/row as fp32r while
halving every DMA, and w2/fv tiles are packed two groups per 128-partition
tile so grouped-stage DMA (~1.2us/group) stays under the Act-engine floor
(~2.1us/group).  PSUM stays fp32; biases fp32.  The scalar engine (SiLU,
67M evals/core) is the critical path at ~95% busy.  The q*k dot product is
folded into the k global stream (per-pair DVE muls) and the final reduce
runs as 16 accumulating ones-matmuls on the PE, so there is no serial
tail.  The (g,p) transpose between stages is a DRAM bounce.  Per-core
output is 512 attention logits; softmax over the 4096 batch is on host.

`reps` unrolls the whole computation R times inside one NEFF (weights
SBUF-resident, double-buffered DRAM f bounce) for steady-state
throughput benchmarking; the correctness path uses reps=1.
"""

import numpy as np

B = 4096
TOTAL_DIM = 4096
G = 64            # groups
D = 64            # group size
H = 512           # hidden
P = 64            # patches
E = 16            # heads
NCORES = 8
BC = B // NCORES  # 512 batch rows per core
NPAIR = P // 2    # 32 patch pairs (global stage)


def _build_nc(reps=1):
    from contextlib import ExitStack
    import concourse.bass as bass
    import concourse.tile as tile
    import concourse.mybir as mybir
    from concourse import bacc

    dt = mybir.dt
    f16 = dt.float16
    f32 = dt.float32
    AF = mybir.ActivationFunctionType

    nc = bacc.Bacc(
        "TRN2",
        target_bir_lowering=False,
        debug=False,
        enable_asserts=False,
        num_devices=NCORES,
    )

    ins = {}
    def din(name, shape, dty):
        ins[name] = nc.dram_tensor(name, shape, dty, kind="ExternalInput").ap()
        return ins[name]

    xq = din("xq", [G * (D + 1), BC], f16)      # rows g*65+d (d<64: x^T), row 64: ones
    xk = din("xk", [G * (D + 1), BC], f16)
    w1q = din("w1q", [G * (D + 1), H], f16)     # rows g*65+d: W1[g,d,:], row 64: b1[g]
    w1k = din("w1k", [G * (D + 1), H], f16)
    # pair-packed W2: row j*128+r, col s*256 + hc*64 + p = W2[2j+s, hc*128+r, p]
    w2q = din("w2q", [NPAIR * 128, 512], f16)
    w2k = din("w2k", [NPAIR * 128, 512], f16)
    wg1 = din("wg1", [128, H], f16)             # Wg1 [64,512] duplicated on both halves
    wg2 = din("wg2", [128, 4 * 32], f16)        # [r, hc*32+e] = Wg2[hc*128+r, e] (e<16, else 0)
    b2q = din("b2q", [64, G], f32)              # col g = b2[g]
    b2k = din("b2k", [64, G], f32)
    bg1p = din("bg1p", [128, 4], f32)           # col hc = bg1[hc*128:(hc+1)*128]
    bg2r = din("bg2r", [128, 1], f32)           # 4x [bg2(16); zeros(16)] along partitions
    ones128 = din("ones128", [128, 1], f16)

    out = nc.dram_tensor("out", [1, BC], f32, kind="ExternalOutput").ap()

    with tile.TileContext(nc) as tc:
        with ExitStack() as ctx:
            ep = ctx.enter_context
            px = ep(tc.tile_pool(name="px", bufs=6))          # x tiles [65,BC]
            pw1 = ep(tc.tile_pool(name="pw1", bufs=6))        # W1 tiles [65,H]
            pw2 = ep(tc.tile_pool(name="pw2", bufs=3))        # W2 pair tiles [128,512]
            phs = ep(tc.tile_pool(name="phs", bufs=4))        # silu'd h [128,1024]
            pfv = ep(tc.tile_pool(name="pfv", bufs=3))        # f pair tiles [128,BC]
            pu = ep(tc.tile_pool(name="pu", bufs=6))          # U tiles [128,BC]
            ph2 = ep(tc.tile_pool(name="ph2", bufs=10))       # silu'd h2 [128,1024]
            pbig = ep(tc.tile_pool(name="pbig", bufs=2))      # qs/ks big [128,16*BC]
            pmisc = ep(tc.tile_pool(name="pmisc", bufs=2))
            pconst = ep(tc.tile_pool(name="pconst", bufs=1))
            # PSUM: psh 3 x 2 banks + psv 2 x 1 bank = 8 banks
            psh = ep(tc.tile_pool(name="psh", bufs=3, space="PSUM"))
            psv = ep(tc.tile_pool(name="psv", bufs=2, space="PSUM"))
            pdram = ep(tc.tile_pool(name="pdram", bufs=2, space="DRAM"))

            def const_tile(src_ap, shape, dty, name):
                t = pconst.tile(shape, dty, name=name, tag=name)
                nc.sync.dma_start(t[:, :], src_ap)
                return t

            wg1_s = const_tile(wg1, [128, H], f16, "wg1s")
            wg2_s = const_tile(wg2, [128, 4 * 32], f16, "wg2s")
            b2q_s = const_tile(b2q, [64, G], f32, "b2qs")
            b2k_s = const_tile(b2k, [64, G], f32, "b2ks")
            bg1_s = const_tile(bg1p, [128, 4], f32, "bg1s")
            bg2_s = const_tile(bg2r, [128, 1], f32, "bg2s")
            one_s = const_tile(ones128, [128, 1], f16, "ones")


            stream_in = {"q": (xq, w1q, w2q, b2q_s), "k": (xk, w1k, w2k, b2k_s)}

            # ================= grouped stage =================
            def grouped(s, fd):
                x_d, w1_d, w2_d, b2_s = stream_in[s]
                for j in range(G // 2):          # group pairs (2j, 2j+1)
                    w2_t = pw2.tile([128, 512], f16, tag="w2")
                    nc.sync.dma_start(w2_t[:, :], w2_d[j * 128:(j + 1) * 128, :])
                    fv = pfv.tile([128, BC], f16, tag="fv")
                    for sgi in range(2):
                        g = 2 * j + sgi
                        x_t = px.tile([D + 1, BC], f16, tag="x")
                        nc.sync.dma_start(x_t[:, :], x_d[g * 65:(g + 1) * 65, :])
                        w1_t = pw1.tile([D + 1, H], f16, tag="w1")
                        nc.sync.dma_start(w1_t[:, :], w1_d[g * 65:(g + 1) * 65, :])
                        v_ps = psv.tile([64, BC], f32, tag="vps")
                        for t in range(2):       # two [128,1024] PSUM tiles = 4 h-chunks
                            hp = psh.tile([128, 1024], f32, tag="hps")
                            for u in range(2):
                                hc = 2 * t + u
                                nc.tensor.matmul(
                                    hp[:, u * 512:(u + 1) * 512],
                                    w1_t[:, hc * 128:(hc + 1) * 128],
                                    x_t[:, :],
                                    start=True, stop=True,
                                )
                            hs_t = phs.tile([128, 1024], f16, tag="hs")
                            nc.scalar.activation(hs_t[:, :], hp[:, :], AF.Silu)
                            for u in range(2):   # GEMM2 accumulation
                                hc = 2 * t + u
                                nc.tensor.matmul(
                                    v_ps[:, :],
                                    w2_t[:, sgi * 256 + hc * 64:sgi * 256 + (hc + 1) * 64],
                                    hs_t[:, u * 512:(u + 1) * 512],
                                    start=(hc == 0), stop=(hc == 3),
                                )
                        nc.vector.tensor_scalar_add(
                            fv[sgi * 64:(sgi + 1) * 64, :], v_ps[:, :],
                            b2_s[:, g:g + 1])
                    nc.sync.dma_start(fd[j * 128:(j + 1) * 128, :], fv[:, :])

            # ================= global stage =================
            def global_stream(s, fd, big, qbig=None):
                fd3 = fd.rearrange("(g p) b -> p g b", p=P)
                for j in range(NPAIR):       # patch pair (2j, 2j+1)
                    u_t = pu.tile([128, BC], f16, tag="u")
                    nc.sync.dma_start(u_t[:, :], fd3[2 * j:2 * j + 2])
                    h2s = []
                    for hc in range(4):
                        h2p = psh.tile([128, 1024], f32, tag="hps")
                        for dp in range(2):
                            nc.tensor.matmul(
                                h2p[:, dp * 512:(dp + 1) * 512],
                                wg1_s[dp * 64:(dp + 1) * 64, hc * 128:(hc + 1) * 128],
                                u_t[dp * 64:(dp + 1) * 64, :],
                                start=True, stop=True,
                                tile_position=(dp * 64, 0),
                            )
                        t = ph2.tile([128, 1024], f16, tag="h2s")
                        nc.scalar.activation(t[:, :], h2p[:, :], AF.Silu,
                                             bias=bg1_s[:, hc:hc + 1])
                        h2s.append(t)
                    for dp in range(2):      # head GEMM per patch (M=32, top 16 real)
                        p_ = 2 * j + dp
                        o_ps = psv.tile([32, BC], f32, tag="vps")
                        for hc in range(4):
                            nc.tensor.matmul(
                                o_ps[:, :],
                                wg2_s[:, hc * 32:(hc + 1) * 32],
                                h2s[hc][:, dp * 512:(dp + 1) * 512],
                                start=(hc == 0), stop=(hc == 3),
                            )
                        # drain into big [128, 16*BC]: partition 32*(p%4), col-block p//4
                        pr, pcb = 32 * (p_ % 4), (p_ // 4) * BC
                        nc.vector.tensor_scalar_add(
                            big[pr:pr + 32, pcb:pcb + BC], o_ps[:, :],
                            bg2_s[pr:pr + 32, 0:1])
                        if qbig is not None:   # fold q*k product into k stream
                            nc.vector.tensor_mul(
                                big[pr:pr + 32, pcb:pcb + BC],
                                qbig[pr:pr + 32, pcb:pcb + BC],
                                big[pr:pr + 32, pcb:pcb + BC])

            for _ in range(reps):
                f_q = pdram.tile([G * P, BC], f16, tag="fq")
                f_k = pdram.tile([G * P, BC], f16, tag="fk")
                grouped("q", f_q)
                grouped("k", f_k)

                qs_big = pbig.tile([128, 16 * BC], f16, tag="qsbig")
                ks_big = pbig.tile([128, 16 * BC], f16, tag="ksbig")
                global_stream("q", f_q, qs_big)
                global_stream("k", f_k, ks_big, qbig=qs_big)

                # ===== logits: reduce q*k product with accumulating matmuls =====
                at_ps = psv.tile([1, BC], f32, tag="vps")
                for c in range(16):
                    nc.tensor.matmul(at_ps[0:1, :], one_s[:, 0:1],
                                     ks_big[:, c * BC:(c + 1) * BC],
                                     start=(c == 0), stop=(c == 15))
                at_s = pmisc.tile([1, BC], f32, tag="at")
                nc.vector.tensor_copy(at_s[0:1, :], at_ps[0:1, :])
                nc.sync.dma_start(out[0:1, :], at_s[0:1, :])

    nc.compile()
    return nc


_NC_CACHE = {}


def _get_nc(reps=1):
    if reps not in _NC_CACHE:
        _NC_CACHE[reps] = _build_nc(reps)
    return _NC_CACHE[reps]


def _prep_inputs(q, k, W1q, b1q, W2q, b2q, W1k, b1k, W2k, b2k, Wg1, bg1, Wg2, bg2):
    f16c = lambda a: np.ascontiguousarray(a, dtype=np.float16)
    f32c = lambda a: np.ascontiguousarray(a, dtype=np.float32)

    def pack_x(x):  # [B, 4096] -> per-core [G*65, BC] with ones row
        shards = []
        for c in range(NCORES):
            xs = np.asarray(x[c * BC:(c + 1) * BC, :], np.float32)
            xt = np.empty((G, D + 1, BC), dtype=np.float16)
            xt[:, :D, :] = xs.T.reshape(G, D, BC).astype(np.float16)
            xt[:, D, :] = 1.0
            shards.append(np.ascontiguousarray(xt.reshape(G * (D + 1), BC)))
        return shards

    def pack_w1(W1, b1):
        w = np.concatenate([np.asarray(W1, np.float32),
                            np.asarray(b1, np.float32)[:, None, :]], axis=1)
        return f16c(w.reshape(G * (D + 1), H))

    def pack_w2(W2):  # [G, 512, 64] -> [G/2*128, 512] pair-packed
        w = np.asarray(W2, np.float32).reshape(G, 4, 128, 64)   # [g, hc, r, p]
        w = w.transpose(0, 2, 1, 3).reshape(G, 128, 256)        # [g, r, hc*64+p]
        w = w.reshape(G // 2, 2, 128, 256).transpose(0, 2, 1, 3)  # [j, r, s, :]
        return f16c(w.reshape(NPAIR * 128, 512))

    xq_s = pack_x(q)
    xk_s = pack_x(k)
    w1q_p = pack_w1(W1q, b1q)
    w1k_p = pack_w1(W1k, b1k)
    w2q_p = pack_w2(W2q)
    w2k_p = pack_w2(W2k)
    b2q_p = f32c(np.asarray(b2q, np.float32).T)                 # [64(P), G]
    b2k_p = f32c(np.asarray(b2k, np.float32).T)

    wg1_p = f16c(np.concatenate([np.asarray(Wg1, np.float32)] * 2, axis=0))  # [128, 512]
    wg2_p = np.zeros((128, 4, 32), dtype=np.float16)
    wg2_p[:, :, :E] = np.asarray(Wg2, np.float32).reshape(4, 128, E).transpose(1, 0, 2).astype(np.float16)
    wg2_p = np.ascontiguousarray(wg2_p.reshape(128, 4 * 32))    # [r, hc*32+e]
    bg1_p = f32c(np.asarray(bg1, np.float32).reshape(4, 128).T)  # [128, 4]
    bg2_p = np.zeros((4, 32), dtype=np.float32)
    bg2_p[:, :E] = np.asarray(bg2, np.float32)
    bg2_p = f32c(bg2_p.reshape(128, 1))
    ones_p = np.ones((128, 1), dtype=np.float16)

    in_maps = []
    for c in range(NCORES):
        in_maps.append({
            "xq": xq_s[c], "xk": xk_s[c],
            "w1q": w1q_p, "w1k": w1k_p,
            "w2q": w2q_p, "w2k": w2k_p,
            "wg1": wg1_p, "wg2": wg2_p,
            "b2q": b2q_p, "b2k": b2k_p,
            "bg1p": bg1_p, "bg2r": bg2_p, "ones128": ones_p,
        })
    return in_maps


def kernel(q, k, W1q, b1q, W2q, b2q, W1k, b1k, W2k, b2k, Wg1, bg1, Wg2, bg2,
           _trace=False, _tracedir=None):
    from concourse.bass_utils import run_bass_kernel_spmd

    in_maps = _prep_inputs(q, k, W1q, b1q, W2q, b2q, W1k, b1k, W2k, b2k,
                           Wg1, bg1, Wg2, bg2)
    nc = _get_nc()
    kw = {}
    if _trace:
        kw = dict(trace=True, tmpdir=_tracedir)
    res = run_bass_kernel_spmd(nc, in_maps, core_ids=list(range(NCORES)), **kw)
    logits = np.concatenate([res.results[c]["out"].reshape(BC)
                             for c in range(NCORES)]).astype(np.float64)
    m = logits.max()
    e = np.exp(logits - m)
    sm = (e / e.sum()).astype(np.float32)
    if _trace:
        kernel._last_trace = res
    return sm


# revision 3
# speedup vs baseline: 8896.9868x; 1.0791x over previous
"""Trainium2 Bass kernel for GroupedKAAttention (fp16 datapath, v3).

Math per batch row b (B=4096 total, 512 per core, data-parallel over 8
NeuronCores, weights replicated):
  xg[b,g,:]  = x[b, g*64:(g+1)*64]                      (G=64 groups, D=64)
  h[b,g,:]   = silu(xg[b,g,:] @ W1[g] + b1[g])          (H=512)
  f[b,g,:]   = h[b,g,:] @ W2[g] + b2[g]                 (P=64 patches)
  h2[b,p,:]  = silu(f[b,:,p] @ Wg1 + bg1)               (contract groups)
  o[b,p,:]   = h2[b,p,:] @ Wg2 + bg2                    (E=16 heads)
  attn[b]    = sum_{p,e} o_q * o_k ;  out = softmax(attn over b)

Design (driven by HW microbenchmarks, not the CoreSim cost model):
- Whole DMA path in fp16 (x, W1, W2, f, Wg1, Wg2, activations): PE fp16
  matmuls run 1 cycle/row like fp32r while halving every DMA; PSUM and
  biases stay fp32.  End-to-end l2 rel err ~2.3e-3 (gate 2e-2).
- The scalar engine (SiLU, 67M evals/core) and PE (~2064 matmuls) are
  co-critical.  All activations are [128,512] (measured 533ns vs 1391ns
  for [128,1024]-to-f16) and bias-free: both MLP stages bake their bias
  into the GEMM as a 65th contraction row of the moving tile (x and the
  per-patch U tile carry a ones row; W1/Wg1 carry the bias row).
- w2/fv tiles pack two groups per 128-partition tile; grouped-stage DMA
  ~2.2us/group sits just at the Act/PE floor.  The (g,p) transpose
  between stages is a DRAM bounce (strided per-patch reads).
- The q*k dot product is folded into the k global stream (per-patch DVE
  muls into the o-buffer) and the final (p,e) reduction runs as 16
  accumulating ones-matmuls on the PE -- no serial tail.
- Per-core output is 512 attention logits; softmax over the 4096-row
  batch happens on host in fp64.

`reps` unrolls the computation R times inside one NEFF (weights stay
SBUF-resident, f bounce double-buffered) for steady-state throughput
benchmarking; the correctness path uses reps=1.
"""

import numpy as np

B = 4096
TOTAL_DIM = 4096
G = 64            # groups
D = 64            # group size
H = 512           # hidden
P = 64            # patches
E = 16            # heads
NCORES = 8
BC = B // NCORES  # 512 batch rows per core
NPAIR = P // 2    # 32 patch pairs (global stage)


def _build_nc(reps=1):
    from contextlib import ExitStack
    import concourse.bass as bass
    import concourse.tile as tile
    import concourse.mybir as mybir
    from concourse import bacc

    dt = mybir.dt
    f16 = dt.float16
    f32 = dt.float32
    AF = mybir.ActivationFunctionType

    nc = bacc.Bacc(
        "TRN2",
        target_bir_lowering=False,
        debug=False,
        enable_asserts=False,
        num_devices=NCORES,
    )

    ins = {}
    def din(name, shape, dty):
        ins[name] = nc.dram_tensor(name, shape, dty, kind="ExternalInput").ap()
        return ins[name]

    xq = din("xq", [G * (D + 1), BC], f16)      # rows g*65+d (d<64: x^T), row 64: ones
    xk = din("xk", [G * (D + 1), BC], f16)
    w1q = din("w1q", [G * (D + 1), H], f16)     # rows g*65+d: W1[g,d,:], row 64: b1[g]
    w1k = din("w1k", [G * (D + 1), H], f16)
    # pair-packed W2: row j*128+r, col s*256 + hc*64 + p = W2[2j+s, hc*128+r, p]
    w2q = din("w2q", [NPAIR * 128, 512], f16)
    w2k = din("w2k", [NPAIR * 128, 512], f16)
    wg1 = din("wg1", [D + 1, H], f16)           # rows 0-63: Wg1, row 64: bg1
    wg2 = din("wg2", [128, 4 * 32], f16)        # [r, hc*32+e] = Wg2[hc*128+r, e] (e<16, else 0)
    b2q = din("b2q", [64, G], f32)              # col g = b2[g]
    b2k = din("b2k", [64, G], f32)
    bg2r = din("bg2r", [128, 1], f32)           # 4x [bg2(16); zeros(16)] along partitions
    ones128 = din("ones128", [128, 1], f16)
    onesbc = din("onesbc", [1, BC], f16)

    out = nc.dram_tensor("out", [1, BC], f32, kind="ExternalOutput").ap()

    with tile.TileContext(nc) as tc:
        with ExitStack() as ctx:
            ep = ctx.enter_context
            px = ep(tc.tile_pool(name="px", bufs=6))          # x tiles [65,BC]
            pw1 = ep(tc.tile_pool(name="pw1", bufs=6))        # W1 tiles [65,H]
            pw2 = ep(tc.tile_pool(name="pw2", bufs=3))        # W2 pair tiles [128,512]
            phs = ep(tc.tile_pool(name="phs", bufs=4))        # silu'd h [128,1024]
            pfv = ep(tc.tile_pool(name="pfv", bufs=3))        # f pair tiles [128,BC]
            pu = ep(tc.tile_pool(name="pu", bufs=6))          # U tiles [128,BC]
            ph2 = ep(tc.tile_pool(name="ph2", bufs=10))       # silu'd h2 [128,1024]
            pbig = ep(tc.tile_pool(name="pbig", bufs=2))      # qs/ks big [128,16*BC]
            pmisc = ep(tc.tile_pool(name="pmisc", bufs=2))
            pconst = ep(tc.tile_pool(name="pconst", bufs=1))
            # PSUM: psh 3 x 2 banks + psv 2 x 1 bank = 8 banks
            psh = ep(tc.tile_pool(name="psh", bufs=3, space="PSUM"))
            psv = ep(tc.tile_pool(name="psv", bufs=2, space="PSUM"))
            pdram = ep(tc.tile_pool(name="pdram", bufs=2, space="DRAM"))

            def const_tile(src_ap, shape, dty, name):
                t = pconst.tile(shape, dty, name=name, tag=name)
                nc.sync.dma_start(t[:, :], src_ap)
                return t

            wg1_s = const_tile(wg1, [D + 1, H], f16, "wg1s")
            wg2_s = const_tile(wg2, [128, 4 * 32], f16, "wg2s")
            b2q_s = const_tile(b2q, [64, G], f32, "b2qs")
            b2k_s = const_tile(b2k, [64, G], f32, "b2ks")
            bg2_s = const_tile(bg2r, [128, 1], f32, "bg2s")
            one_s = const_tile(ones128, [128, 1], f16, "ones")
            ones_bc = const_tile(onesbc, [1, BC], f16, "onesbc")


            stream_in = {"q": (xq, w1q, w2q, b2q_s), "k": (xk, w1k, w2k, b2k_s)}

            # ================= grouped stage =================
            def grouped(s, fd):
                x_d, w1_d, w2_d, b2_s = stream_in[s]
                for j in range(G // 2):          # group pairs (2j, 2j+1)
                    w2_t = pw2.tile([128, 512], f16, tag="w2")
                    nc.sync.dma_start(w2_t[:, :], w2_d[j * 128:(j + 1) * 128, :])
                    fv = pfv.tile([128, BC], f16, tag="fv")
                    for sgi in range(2):
                        g = 2 * j + sgi
                        x_t = px.tile([D + 1, BC], f16, tag="x")
                        nc.sync.dma_start(x_t[:, :], x_d[g * 65:(g + 1) * 65, :])
                        w1_t = pw1.tile([D + 1, H], f16, tag="w1")
                        nc.sync.dma_start(w1_t[:, :], w1_d[g * 65:(g + 1) * 65, :])
                        v_ps = psv.tile([64, BC], f32, tag="vps")
                        for t in range(2):       # two [128,1024] PSUM tiles = 4 h-chunks
                            hp = psh.tile([128, 1024], f32, tag="hps")
                            for u in range(2):
                                hc = 2 * t + u
                                nc.tensor.matmul(
                                    hp[:, u * 512:(u + 1) * 512],
                                    w1_t[:, hc * 128:(hc + 1) * 128],
                                    x_t[:, :],
                                    start=True, stop=True,
                                )
                            for u in range(2):   # 512-wide acts + GEMM2 accumulation
                                hc = 2 * t + u
                                hs_t = phs.tile([128, 512], f16, tag="hs")
                                nc.scalar.activation(
                                    hs_t[:, :], hp[:, u * 512:(u + 1) * 512], AF.Silu)
                                nc.tensor.matmul(
                                    v_ps[:, :],
                                    w2_t[:, sgi * 256 + hc * 64:sgi * 256 + (hc + 1) * 64],
                                    hs_t[:, :],
                                    start=(hc == 0), stop=(hc == 3),
                                )
                        nc.vector.tensor_scalar_add(
                            fv[sgi * 64:(sgi + 1) * 64, :], v_ps[:, :],
                            b2_s[:, g:g + 1])
                    nc.sync.dma_start(fd[j * 128:(j + 1) * 128, :], fv[:, :])

            # ================= global stage =================
            # per patch p: u_p [65, BC] (64 groups + ones row), bg1 rides as
            # wg1_s row 64 so the 512-wide acts are bias-free
            def global_stream(s, fd, big, qbig=None):
                fd3 = fd.rearrange("(g p) b -> p g b", p=P)
                for p_ in range(P):
                    u_t = pu.tile([D + 1, BC], f16, tag="u")
                    nc.sync.dma_start(u_t[0:D, :], fd3[p_])
                    nc.vector.tensor_copy(u_t[D:D + 1, :], ones_bc[0:1, :])
                    h2s = []
                    for t in range(2):
                        h2p = psh.tile([128, 1024], f32, tag="hps")
                        for u in range(2):
                            hc = 2 * t + u
                            nc.tensor.matmul(
                                h2p[:, u * 512:(u + 1) * 512],
                                wg1_s[:, hc * 128:(hc + 1) * 128],
                                u_t[:, :],
                                start=True, stop=True,
                            )
                        for u in range(2):
                            ht = ph2.tile([128, 512], f16, tag="h2s")
                            nc.scalar.activation(
                                ht[:, :], h2p[:, u * 512:(u + 1) * 512], AF.Silu)
                            h2s.append(ht)
                    o_ps = psv.tile([32, BC], f32, tag="vps")
                    for hc in range(4):      # head GEMM (M=32, top 16 real)
                        nc.tensor.matmul(
                            o_ps[:, :],
                            wg2_s[:, hc * 32:(hc + 1) * 32],
                            h2s[hc][:, :],
                            start=(hc == 0), stop=(hc == 3),
                        )
                    # drain into big [128, 16*BC]: partition 32*(p%4), col-block p//4
                    pr, pcb = 32 * (p_ % 4), (p_ // 4) * BC
                    nc.vector.tensor_scalar_add(
                        big[pr:pr + 32, pcb:pcb + BC], o_ps[:, :],
                        bg2_s[pr:pr + 32, 0:1])
                    if qbig is not None:   # fold q*k product into k stream
                        nc.vector.tensor_mul(
                            big[pr:pr + 32, pcb:pcb + BC],
                            qbig[pr:pr + 32, pcb:pcb + BC],
                            big[pr:pr + 32, pcb:pcb + BC])

            for _ in range(reps):
                f_q = pdram.tile([G * P, BC], f16, tag="fq")
                f_k = pdram.tile([G * P, BC], f16, tag="fk")
                grouped("q", f_q)
                grouped("k", f_k)

                qs_big = pbig.tile([128, 16 * BC], f16, tag="qsbig")
                ks_big = pbig.tile([128, 16 * BC], f16, tag="ksbig")
                global_stream("q", f_q, qs_big)
                global_stream("k", f_k, ks_big, qbig=qs_big)

                # ===== logits: reduce q*k product with accumulating matmuls =====
                at_ps = psv.tile([1, BC], f32, tag="vps")
                for c in range(16):
                    nc.tensor.matmul(at_ps[0:1, :], one_s[:, 0:1],
                                     ks_big[:, c * BC:(c + 1) * BC],
                                     start=(c == 0), stop=(c == 15))
                at_s = pmisc.tile([1, BC], f32, tag="at")
                nc.vector.tensor_copy(at_s[0:1, :], at_ps[0:1, :])
                nc.sync.dma_start(out[0:1, :], at_s[0:1, :])

    nc.compile()
    return nc


_NC_CACHE = {}


def _get_nc(reps=1):
    if reps not in _NC_CACHE:
        _NC_CACHE[reps] = _build_nc(reps)
    return _NC_CACHE[reps]


def _prep_inputs(q, k, W1q, b1q, W2q, b2q, W1k, b1k, W2k, b2k, Wg1, bg1, Wg2, bg2):
    f16c = lambda a: np.ascontiguousarray(a, dtype=np.float16)
    f32c = lambda a: np.ascontiguousarray(a, dtype=np.float32)

    def pack_x(x):  # [B, 4096] -> per-core [G*65, BC] with ones row
        shards = []
        for c in range(NCORES):
            xs = np.asarray(x[c * BC:(c + 1) * BC, :], np.float32)
            xt = np.empty((G, D + 1, BC), dtype=np.float16)
            xt[:, :D, :] = xs.T.reshape(G, D, BC).astype(np.float16)
            xt[:, D, :] = 1.0
            shards.append(np.ascontiguousarray(xt.reshape(G * (D + 1), BC)))
        return shards

    def pack_w1(W1, b1):
        w = np.concatenate([np.asarray(W1, np.float32),
                            np.asarray(b1, np.float32)[:, None, :]], axis=1)
        return f16c(w.reshape(G * (D + 1), H))

    def pack_w2(W2):  # [G, 512, 64] -> [G/2*128, 512] pair-packed
        w = np.asarray(W2, np.float32).reshape(G, 4, 128, 64)   # [g, hc, r, p]
        w = w.transpose(0, 2, 1, 3).reshape(G, 128, 256)        # [g, r, hc*64+p]
        w = w.reshape(G // 2, 2, 128, 256).transpose(0, 2, 1, 3)  # [j, r, s, :]
        return f16c(w.reshape(NPAIR * 128, 512))

    xq_s = pack_x(q)
    xk_s = pack_x(k)
    w1q_p = pack_w1(W1q, b1q)
    w1k_p = pack_w1(W1k, b1k)
    w2q_p = pack_w2(W2q)
    w2k_p = pack_w2(W2k)
    b2q_p = f32c(np.asarray(b2q, np.float32).T)                 # [64(P), G]
    b2k_p = f32c(np.asarray(b2k, np.float32).T)

    wg1_p = f16c(np.concatenate([np.asarray(Wg1, np.float32),
                                 np.asarray(bg1, np.float32)[None, :]], axis=0))  # [65, 512]
    wg2_p = np.zeros((128, 4, 32), dtype=np.float16)
    wg2_p[:, :, :E] = np.asarray(Wg2, np.float32).reshape(4, 128, E).transpose(1, 0, 2).astype(np.float16)
    wg2_p = np.ascontiguousarray(wg2_p.reshape(128, 4 * 32))    # [r, hc*32+e]
    bg2_p = np.zeros((4, 32), dtype=np.float32)
    bg2_p[:, :E] = np.asarray(bg2, np.float32)
    bg2_p = f32c(bg2_p.reshape(128, 1))
    ones_p = np.ones((128, 1), dtype=np.float16)

    in_maps = []
    for c in range(NCORES):
        in_maps.append({
            "xq": xq_s[c], "xk": xk_s[c],
            "w1q": w1q_p, "w1k": w1k_p,
            "w2q": w2q_p, "w2k": w2k_p,
            "wg1": wg1_p, "wg2": wg2_p,
            "b2q": b2q_p, "b2k": b2k_p,
            "bg2r": bg2_p, "ones128": ones_p,
            "onesbc": np.ones((1, BC), dtype=np.float16),
        })
    return in_maps


def kernel(q, k, W1q, b1q, W2q, b2q, W1k, b1k, W2k, b2k, Wg1, bg1, Wg2, bg2,
           _trace=False, _tracedir=None):
    from concourse.bass_utils import run_bass_kernel_spmd

    in_maps = _prep_inputs(q, k, W1q, b1q, W2q, b2q, W1k, b1k, W2k, b2k,
                           Wg1, bg1, Wg2, bg2)
    nc = _get_nc()
    kw = {}
    if _trace:
        kw = dict(trace=True, tmpdir=_tracedir)
    res = run_bass_kernel_spmd(nc, in_maps, core_ids=list(range(NCORES)), **kw)
    logits = np.concatenate([res.results[c]["out"].reshape(BC)
                             for c in range(NCORES)]).astype(np.float64)
    m = logits.max()
    e = np.exp(logits - m)
    sm = (e / e.sum()).astype(np.float32)
    if _trace:
        kernel._last_trace = res
    return sm


# revision 4
# speedup vs baseline: 8970.3229x; 1.0082x over previous
"""Trainium2 Bass kernel for GroupedKAAttention (fp16 datapath, v3).

Math per batch row b (B=4096 total, 512 per core, data-parallel over 8
NeuronCores, weights replicated):
  xg[b,g,:]  = x[b, g*64:(g+1)*64]                      (G=64 groups, D=64)
  h[b,g,:]   = silu(xg[b,g,:] @ W1[g] + b1[g])          (H=512)
  f[b,g,:]   = h[b,g,:] @ W2[g] + b2[g]                 (P=64 patches)
  h2[b,p,:]  = silu(f[b,:,p] @ Wg1 + bg1)               (contract groups)
  o[b,p,:]   = h2[b,p,:] @ Wg2 + bg2                    (E=16 heads)
  attn[b]    = sum_{p,e} o_q * o_k ;  out = softmax(attn over b)

Design (driven by HW microbenchmarks, not the CoreSim cost model):
- Whole DMA path in fp16 (x, W1, W2, f, Wg1, Wg2, activations): PE fp16
  matmuls run 1 cycle/row like fp32r while halving every DMA; PSUM and
  biases stay fp32.  End-to-end l2 rel err ~2.3e-3 (gate 2e-2).
- The scalar engine (SiLU, 67M evals/core) and PE (~2064 matmuls) are
  co-critical.  All activations are [128,512] (measured 533ns vs 1391ns
  for [128,1024]-to-f16) and bias-free: both MLP stages bake their bias
  into the GEMM as a 65th contraction row of the moving tile (x and the
  per-patch U tile carry a ones row; W1/Wg1 carry the bias row).
- w2/fv tiles pack two groups per 128-partition tile; grouped-stage DMA
  ~2.2us/group sits just at the Act/PE floor.  The (g,p) transpose
  between stages is a DRAM bounce (strided per-patch reads).
- The q*k dot product is folded into the k global stream (per-patch DVE
  muls into the o-buffer) and the final (p,e) reduction runs as 16
  accumulating ones-matmuls on the PE -- no serial tail.
- Per-core output is 512 attention logits; softmax over the 4096-row
  batch happens on host in fp64.

`reps` unrolls the computation R times inside one NEFF (weights stay
SBUF-resident, f bounce double-buffered) for steady-state throughput
benchmarking; the correctness path uses reps=1.
"""

import numpy as np

B = 4096
TOTAL_DIM = 4096
G = 64            # groups
D = 64            # group size
H = 512           # hidden
P = 64            # patches
E = 16            # heads
NCORES = 8
BC = B // NCORES  # 512 batch rows per core
NPAIR = P // 2    # 32 patch pairs (global stage)


def _build_nc(reps=1):
    from contextlib import ExitStack
    import concourse.bass as bass
    import concourse.tile as tile
    import concourse.mybir as mybir
    from concourse import bacc

    dt = mybir.dt
    f16 = dt.float16
    f32 = dt.float32
    AF = mybir.ActivationFunctionType

    nc = bacc.Bacc(
        "TRN2",
        target_bir_lowering=False,
        debug=False,
        enable_asserts=False,
        num_devices=NCORES,
    )

    ins = {}
    def din(name, shape, dty):
        ins[name] = nc.dram_tensor(name, shape, dty, kind="ExternalInput").ap()
        return ins[name]

    xq = din("xq", [G * (D + 1), BC], f16)      # rows g*65+d (d<64: x^T), row 64: ones
    xk = din("xk", [G * (D + 1), BC], f16)
    w1q = din("w1q", [G * (D + 1), H], f16)     # rows g*65+d: W1[g,d,:], row 64: b1[g]
    w1k = din("w1k", [G * (D + 1), H], f16)
    # pair-packed W2: row j*128+r, col s*256 + hc*64 + p = W2[2j+s, hc*128+r, p]
    w2q = din("w2q", [NPAIR * 128, 512], f16)
    w2k = din("w2k", [NPAIR * 128, 512], f16)
    wg1 = din("wg1", [D + 1, H], f16)           # rows 0-63: Wg1, row 64: bg1
    wg2 = din("wg2", [128, 4 * 32], f16)        # [r, hc*32+e] = Wg2[hc*128+r, e] (e<16, else 0)
    b2q = din("b2q", [64, G], f32)              # col g = b2[g]
    b2k = din("b2k", [64, G], f32)
    bg2r = din("bg2r", [128, 1], f32)           # 4x [bg2(16); zeros(16)] along partitions
    ones128 = din("ones128", [128, 1], f16)
    onesbc = din("onesbc", [1, BC], f16)

    out = nc.dram_tensor("out", [1, BC], f32, kind="ExternalOutput").ap()

    with tile.TileContext(nc) as tc:
        with ExitStack() as ctx:
            ep = ctx.enter_context
            px = ep(tc.tile_pool(name="px", bufs=8))          # x tiles [65,BC]
            pw1 = ep(tc.tile_pool(name="pw1", bufs=8))        # W1 tiles [65,H]
            pw2 = ep(tc.tile_pool(name="pw2", bufs=3))        # W2 pair tiles [128,512]
            phs = ep(tc.tile_pool(name="phs", bufs=8))        # silu'd h [128,1024]
            pfv = ep(tc.tile_pool(name="pfv", bufs=3))        # f pair tiles [128,BC]
            pu = ep(tc.tile_pool(name="pu", bufs=6))          # U tiles [128,BC]
            ph2 = ep(tc.tile_pool(name="ph2", bufs=10))       # silu'd h2 [128,1024]
            pbig = ep(tc.tile_pool(name="pbig", bufs=2))      # qs/ks big [128,16*BC]
            pmisc = ep(tc.tile_pool(name="pmisc", bufs=2))
            pconst = ep(tc.tile_pool(name="pconst", bufs=1))
            # PSUM: psh 3 x 2 banks + psv 2 x 1 bank = 8 banks
            psh = ep(tc.tile_pool(name="psh", bufs=3, space="PSUM"))
            psv = ep(tc.tile_pool(name="psv", bufs=2, space="PSUM"))
            pdram = ep(tc.tile_pool(name="pdram", bufs=4, space="DRAM"))

            def const_tile(src_ap, shape, dty, name):
                t = pconst.tile(shape, dty, name=name, tag=name)
                nc.sync.dma_start(t[:, :], src_ap)
                return t

            wg1_s = const_tile(wg1, [D + 1, H], f16, "wg1s")
            wg2_s = const_tile(wg2, [128, 4 * 32], f16, "wg2s")
            b2q_s = const_tile(b2q, [64, G], f32, "b2qs")
            b2k_s = const_tile(b2k, [64, G], f32, "b2ks")
            bg2_s = const_tile(bg2r, [128, 1], f32, "bg2s")
            one_s = const_tile(ones128, [128, 1], f16, "ones")
            ones_bc = const_tile(onesbc, [1, BC], f16, "onesbc")


            stream_in = {"q": (xq, w1q, w2q, b2q_s), "k": (xk, w1k, w2k, b2k_s)}

            # ================= grouped stage =================
            def grouped(s, fd):
                x_d, w1_d, w2_d, b2_s = stream_in[s]
                for j in range(G // 2):          # group pairs (2j, 2j+1)
                    w2_t = pw2.tile([128, 512], f16, tag="w2")
                    nc.gpsimd.dma_start(w2_t[:, :], w2_d[j * 128:(j + 1) * 128, :])
                    fv = pfv.tile([128, BC], f16, tag="fv")
                    for sgi in range(2):
                        g = 2 * j + sgi
                        x_t = px.tile([D + 1, BC], f16, tag="x")
                        nc.sync.dma_start(x_t[:, :], x_d[g * 65:(g + 1) * 65, :])
                        w1_t = pw1.tile([D + 1, H], f16, tag="w1")
                        nc.sync.dma_start(w1_t[:, :], w1_d[g * 65:(g + 1) * 65, :])
                        v_ps = psv.tile([64, BC], f32, tag="vps")
                        for t in range(2):       # two [128,1024] PSUM tiles = 4 h-chunks
                            hp = psh.tile([128, 1024], f32, tag="hps")
                            for u in range(2):
                                hc = 2 * t + u
                                nc.tensor.matmul(
                                    hp[:, u * 512:(u + 1) * 512],
                                    w1_t[:, hc * 128:(hc + 1) * 128],
                                    x_t[:, :],
                                    start=True, stop=True,
                                )
                            for u in range(2):   # 512-wide acts + GEMM2 accumulation
                                hc = 2 * t + u
                                hs_t = phs.tile([128, 512], f16, tag="hs")
                                nc.scalar.activation(
                                    hs_t[:, :], hp[:, u * 512:(u + 1) * 512], AF.Silu)
                                nc.tensor.matmul(
                                    v_ps[:, :],
                                    w2_t[:, sgi * 256 + hc * 64:sgi * 256 + (hc + 1) * 64],
                                    hs_t[:, :],
                                    start=(hc == 0), stop=(hc == 3),
                                )
                        nc.vector.tensor_scalar_add(
                            fv[sgi * 64:(sgi + 1) * 64, :], v_ps[:, :],
                            b2_s[:, g:g + 1])
                    nc.gpsimd.dma_start(fd[j * 128:(j + 1) * 128, :], fv[:, :])

            # ================= global stage =================
            # per patch p: u_p [65, BC] (64 groups + ones row), bg1 rides as
            # wg1_s row 64 so the 512-wide acts are bias-free
            def global_stream(s, fd, big, qbig=None):
                fd3 = fd.rearrange("(g p) b -> p g b", p=P)
                for p_ in range(P):
                    u_t = pu.tile([D + 1, BC], f16, tag="u")
                    nc.sync.dma_start(u_t[0:D, :], fd3[p_])
                    nc.vector.tensor_copy(u_t[D:D + 1, :], ones_bc[0:1, :])
                    h2s = []
                    for t in range(2):
                        h2p = psh.tile([128, 1024], f32, tag="hps")
                        for u in range(2):
                            hc = 2 * t + u
                            nc.tensor.matmul(
                                h2p[:, u * 512:(u + 1) * 512],
                                wg1_s[:, hc * 128:(hc + 1) * 128],
                                u_t[:, :],
                                start=True, stop=True,
                            )
                        for u in range(2):
                            ht = ph2.tile([128, 512], f16, tag="h2s")
                            nc.scalar.activation(
                                ht[:, :], h2p[:, u * 512:(u + 1) * 512], AF.Silu)
                            h2s.append(ht)
                    o_ps = psv.tile([32, BC], f32, tag="vps")
                    for hc in range(4):      # head GEMM (M=32, top 16 real)
                        nc.tensor.matmul(
                            o_ps[:, :],
                            wg2_s[:, hc * 32:(hc + 1) * 32],
                            h2s[hc][:, :],
                            start=(hc == 0), stop=(hc == 3),
                        )
                    # drain into big [128, 16*BC]: partition 32*(p%4), col-block p//4
                    pr, pcb = 32 * (p_ % 4), (p_ // 4) * BC
                    nc.vector.tensor_scalar_add(
                        big[pr:pr + 32, pcb:pcb + BC], o_ps[:, :],
                        bg2_s[pr:pr + 32, 0:1])
                    if qbig is not None:   # fold q*k product into k stream
                        nc.vector.tensor_mul(
                            big[pr:pr + 32, pcb:pcb + BC],
                            qbig[pr:pr + 32, pcb:pcb + BC],
                            big[pr:pr + 32, pcb:pcb + BC])

            for _ in range(reps):
                f_q = pdram.tile([G * P, BC], f16, tag="fq")
                f_k = pdram.tile([G * P, BC], f16, tag="fk")
                grouped("q", f_q)
                grouped("k", f_k)

                qs_big = pbig.tile([128, 16 * BC], f16, tag="qsbig")
                ks_big = pbig.tile([128, 16 * BC], f16, tag="ksbig")
                global_stream("q", f_q, qs_big)
                global_stream("k", f_k, ks_big, qbig=qs_big)

                # ===== logits: reduce q*k product with accumulating matmuls =====
                at_ps = psv.tile([1, BC], f32, tag="vps")
                for c in range(16):
                    nc.tensor.matmul(at_ps[0:1, :], one_s[:, 0:1],
                                     ks_big[:, c * BC:(c + 1) * BC],
                                     start=(c == 0), stop=(c == 15))
                at_s = pmisc.tile([1, BC], f32, tag="at")
                nc.vector.tensor_copy(at_s[0:1, :], at_ps[0:1, :])
                nc.sync.dma_start(out[0:1, :], at_s[0:1, :])

    nc.compile()
    return nc


_NC_CACHE = {}


def _get_nc(reps=1):
    if reps not in _NC_CACHE:
        _NC_CACHE[reps] = _build_nc(reps)
    return _NC_CACHE[reps]


def _prep_inputs(q, k, W1q, b1q, W2q, b2q, W1k, b1k, W2k, b2k, Wg1, bg1, Wg2, bg2):
    f16c = lambda a: np.ascontiguousarray(a, dtype=np.float16)
    f32c = lambda a: np.ascontiguousarray(a, dtype=np.float32)

    def pack_x(x):  # [B, 4096] -> per-core [G*65, BC] with ones row
        shards = []
        for c in range(NCORES):
            xs = np.asarray(x[c * BC:(c + 1) * BC, :], np.float32)
            xt = np.empty((G, D + 1, BC), dtype=np.float16)
            xt[:, :D, :] = xs.T.reshape(G, D, BC).astype(np.float16)
            xt[:, D, :] = 1.0
            shards.append(np.ascontiguousarray(xt.reshape(G * (D + 1), BC)))
        return shards

    def pack_w1(W1, b1):
        w = np.concatenate([np.asarray(W1, np.float32),
                            np.asarray(b1, np.float32)[:, None, :]], axis=1)
        return f16c(w.reshape(G * (D + 1), H))

    def pack_w2(W2):  # [G, 512, 64] -> [G/2*128, 512] pair-packed
        w = np.asarray(W2, np.float32).reshape(G, 4, 128, 64)   # [g, hc, r, p]
        w = w.transpose(0, 2, 1, 3).reshape(G, 128, 256)        # [g, r, hc*64+p]
        w = w.reshape(G // 2, 2, 128, 256).transpose(0, 2, 1, 3)  # [j, r, s, :]
        return f16c(w.reshape(NPAIR * 128, 512))

    xq_s = pack_x(q)
    xk_s = pack_x(k)
    w1q_p = pack_w1(W1q, b1q)
    w1k_p = pack_w1(W1k, b1k)
    w2q_p = pack_w2(W2q)
    w2k_p = pack_w2(W2k)
    b2q_p = f32c(np.asarray(b2q, np.float32).T)                 # [64(P), G]
    b2k_p = f32c(np.asarray(b2k, np.float32).T)

    wg1_p = f16c(np.concatenate([np.asarray(Wg1, np.float32),
                                 np.asarray(bg1, np.float32)[None, :]], axis=0))  # [65, 512]
    wg2_p = np.zeros((128, 4, 32), dtype=np.float16)
    wg2_p[:, :, :E] = np.asarray(Wg2, np.float32).reshape(4, 128, E).transpose(1, 0, 2).astype(np.float16)
    wg2_p = np.ascontiguousarray(wg2_p.reshape(128, 4 * 32))    # [r, hc*32+e]
    bg2_p = np.zeros((4, 32), dtype=np.float32)
    bg2_p[:, :E] = np.asarray(bg2, np.float32)
    bg2_p = f32c(bg2_p.reshape(128, 1))
    ones_p = np.ones((128, 1), dtype=np.float16)

    in_maps = []
    for c in range(NCORES):
        in_maps.append({
            "xq": xq_s[c], "xk": xk_s[c],
            "w1q": w1q_p, "w1k": w1k_p,
            "w2q": w2q_p, "w2k": w2k_p,
            "wg1": wg1_p, "wg2": wg2_p,
            "b2q": b2q_p, "b2k": b2k_p,
            "bg2r": bg2_p, "ones128": ones_p,
            "onesbc": np.ones((1, BC), dtype=np.float16),
        })
    return in_maps


def kernel(q, k, W1q, b1q, W2q, b2q, W1k, b1k, W2k, b2k, Wg1, bg1, Wg2, bg2,
           _trace=False, _tracedir=None):
    from concourse.bass_utils import run_bass_kernel_spmd

    in_maps = _prep_inputs(q, k, W1q, b1q, W2q, b2q, W1k, b1k, W2k, b2k,
                           Wg1, bg1, Wg2, bg2)
    nc = _get_nc()
    kw = {}
    if _trace:
        kw = dict(trace=True, tmpdir=_tracedir)
    res = run_bass_kernel_spmd(nc, in_maps, core_ids=list(range(NCORES)), **kw)
    logits = np.concatenate([res.results[c]["out"].reshape(BC)
                             for c in range(NCORES)]).astype(np.float64)
    m = logits.max()
    e = np.exp(logits - m)
    sm = (e / e.sum()).astype(np.float32)
    if _trace:
        kernel._last_trace = res
    return sm
